# revision 1
# baseline (speedup 1.0000x reference)
"""CliffordLayerNorm Trainium2 kernel.

x: [16, 4096, 1024] fp32. Each row's 1024 features = 4 blocks of 256
multivector components; components are grouped into 9 grades by popcount of
their index within the block.  Per (token, block, grade): mean/var, then
out = (x - mean) * w[g] * rsqrt(var + eps) + b[g].

Strategy (per NeuronCore, data-parallel over tokens across 8 cores):
  1. DMA in token-major tiles [128 tok, 1024 feat].
  2. PE-transpose each 128x128 chunk into PSUM (feature-major).
  3. ACT copies PSUM -> SBUF (x_T) and squares PSUM -> SBUF (sq_T).
  4. PE matmuls against a grade-membership matrix (entries 1/count) give
     per-(block,grade) mean and mean-of-squares directly: PSUM [72, T].
  5. Small DVE/ACT/GPSIMD ops produce rstd and (b/w - mean*rstd) stats.
  6. PE scatter-matmuls (stats as stationary, w-scaled grade indicator as
     moving) expand stats back to per-element scale A and shift B in
     token-major layout.
  7. DVE: out = x * A + B, DMA out.
"""

import os
import sys

if "/opt/trn_rl_repo" not in sys.path:
    sys.path.insert(0, "/opt/trn_rl_repo")

import numpy as np

BLOCK_BITS = 8
MV = 256
NG = 9
NB = 4
D = 1024
EPS = 1e-5
N_CORES = 8
TOTAL_TOKENS = 16 * 4096
TOK_PER_CORE = TOTAL_TOKENS // N_CORES  # 8192

GROUP_T = 256          # tokens per stats group
TILE_T = 128           # tokens per tile (partition dim)

# Matmul operand dtype: float32r runs at 1 cycle/row (vs 4 for float32) on
# the PE at N>=256; accumulation stays fp32 in PSUM.
USE_F32R = True


def _grade(m):
    return bin(m).count("1")


def _build_consts():
    import math
    counts = np.array([math.comb(8, g) for g in range(NG)], dtype=np.float32)

    # G_mean[h][i, b*9+g] = 1/count_g  for chunk h (features 128h..128h+127)
    gmean = np.zeros((8, 128, 36), dtype=np.float32)
    for h in range(8):
        b = h // 2
        for i in range(128):
            m = (h % 2) * 128 + i
            g = _grade(m)
            gmean[h, i, b * 9 + g] = 1.0 / counts[g]

    # G01[b*9+g, c] = 1 if feature c belongs to (block b, grade g)
    g01 = np.zeros((36, D), dtype=np.float32)
    for c in range(D):
        b = c // MV
        g = _grade(c % MV)
        g01[b * 9 + g, c] = 1.0

    # rstd mask: count-1 grades (0 and 8) have centered value exactly 0 in
    # the reference, so any scale works -- force rstd=0 there to avoid
    # amplifying f32r rounding by rsqrt(eps).
    mask = np.ones((36, 1), dtype=np.float32)
    for b in range(NB):
        mask[b * 9 + 0, 0] = 0.0
        mask[b * 9 + 8, 0] = 0.0
    return gmean, g01, mask


def build_nc(tok_per_core=TOK_PER_CORE, use_f32r=USE_F32R, loop_reps=1):
    import concourse.bass as bass
    import concourse.tile as tile
    from concourse import bacc, mybir

    f32 = mybir.dt.float32
    f32r = mybir.dt.float32r
    AF = mybir.ActivationFunctionType
    ALU = mybir.AluOpType

    fmm = f32r if use_f32r else f32
    fst = mybir.dt.bfloat16 if use_f32r else f32   # stats-matmul operand dtype

    gmean_np, g01_np, mask_np = _build_consts()
    n_groups = tok_per_core // GROUP_T
    assert tok_per_core % GROUP_T == 0

    nc = bacc.Bacc()
    x_d = nc.dram_tensor("x", [tok_per_core, D], f32, kind="ExternalInput")
    w_d = nc.dram_tensor("weight", [NG], f32, kind="ExternalInput")
    b_d = nc.dram_tensor("bias", [NG], f32, kind="ExternalInput")
    out_d = nc.dram_tensor("out", [tok_per_core, D], f32, kind="ExternalOutput")

    gmean_dram = nc.inline_tensor(gmean_np, name="gmean_const")
    g01_dram = nc.inline_tensor(g01_np, name="g01_const")
    ident_dram = nc.inline_tensor(np.eye(128, dtype=np.float32), name="ident_const")
    mask_dram = nc.inline_tensor(mask_np, name="mask_const")

    from contextlib import ExitStack

    with tile.TileContext(nc) as tc, ExitStack() as ctx:
        consts = ctx.enter_context(tc.tile_pool(name="consts", bufs=1))
        xg_pool = ctx.enter_context(tc.tile_pool(name="xg", bufs=10))
        xt_pool = ctx.enter_context(tc.tile_pool(name="xt", bufs=4))
        sqt_pool = ctx.enter_context(tc.tile_pool(name="sqt", bufs=4))
        tmp_pool = ctx.enter_context(tc.tile_pool(name="tmp", bufs=6))
        small_pool = ctx.enter_context(tc.tile_pool(name="small", bufs=4))
        ps_xt = ctx.enter_context(tc.tile_pool(name="ps_xt", bufs=2, space="PSUM"))
        ps_stats = ctx.enter_context(tc.tile_pool(name="ps_st", bufs=2, space="PSUM"))
        ps_a = ctx.enter_context(tc.tile_pool(name="ps_a", bufs=2, space="PSUM"))
        ps_b = ctx.enter_context(tc.tile_pool(name="ps_b", bufs=2, space="PSUM"))

        # ---- constants into SBUF ----
        # All const DMAs go through gpsimd (SWDGE, single queue -> single
        # semaphore) so downstream compute needs at most one new wait.
        ident = consts.tile([128, 128], f32)
        nc.gpsimd.dma_start(out=ident, in_=ident_dram[:])

        gmean_f = consts.tile([128, 8, 36], f32)
        nc.gpsimd.dma_start(out=gmean_f, in_=gmean_dram[:].rearrange("h p c -> p h c"))

        g01_sb = consts.tile([36, D], f32)
        nc.gpsimd.dma_start(out=g01_sb, in_=g01_dram[:])

        # weight/bias broadcast to 36 partitions: partition p = b*9+g reads w[g]
        w36 = consts.tile([36, 1], f32)
        b36 = consts.tile([36, 1], f32)
        wap = w_d[:]
        bap = b_d[:]
        nc.gpsimd.dma_start(
            out=w36, in_=bass.AP(tensor=wap.tensor, offset=wap.offset,
                                 ap=[[0, NB]] + list(wap.ap)))
        nc.gpsimd.dma_start(
            out=b36, in_=bass.AP(tensor=bap.tensor, offset=bap.offset,
                                 ap=[[0, NB]] + list(bap.ap)))

        mask36 = consts.tile([36, 1], f32)
        nc.gpsimd.dma_start(out=mask36, in_=mask_dram[:])
        # eps + 1e38*(1-mask): count-1 grades get a huge bias so the fused
        # abs-rsqrt returns ~1e-19 (i.e. rstd ~= 0) for them
        eps36 = consts.tile([36, 1], f32)
        nc.vector.tensor_scalar(
            out=eps36, in0=mask36, scalar1=-1e38, scalar2=1e38 + EPS,
            op0=ALU.mult, op1=ALU.add)
        gmean_sb = consts.tile([128, 8, 36], fst)
        nc.vector.tensor_scalar_mul(gmean_sb, gmean_f, 1.0)
        rw36 = consts.tile([36, 1], f32)
        nc.vector.reciprocal(rw36, w36)
        # GA[bg, c] = w[g(c)] * indicator; ga_mask additionally zeroes
        # count-1 grades (their centered value is exactly 0 in the reference)
        ga_sb = consts.tile([36, D], fmm)
        nc.vector.tensor_scalar_mul(ga_sb, g01_sb, w36)
        w36m = consts.tile([36, 1], f32)
        nc.vector.tensor_scalar_mul(w36m, w36, mask36)
        ga_mask = consts.tile([36, D], fmm)
        nc.vector.tensor_scalar_mul(ga_mask, g01_sb, w36m)
        bw36 = consts.tile([36, 1], f32)   # b/w  (rw36 is 2 DVE insts old here)
        nc.vector.tensor_scalar_mul(bw36, b36, rw36)

        # ---- main loop ----
        rep_ctx = tc.For_i(0, loop_reps, 1) if loop_reps > 1 else None
        if rep_ctx is not None:
            rep_ctx.__enter__()
        for gi in range(n_groups):
            tok0 = gi * GROUP_T
            x_group = xg_pool.tile([128, 2, D], f32)
            nc.sync.dma_start(
                out=x_group,
                in_=x_d[tok0:tok0 + GROUP_T, :].rearrange("(j p) d -> p j d", p=128),
            )

            xT = xt_pool.tile([128, 8, GROUP_T], fst)
            sqT = sqt_pool.tile([128, 8, GROUP_T], fst)

            for j in range(2):
                for half in range(2):
                    xt_ps = ps_xt.tile([128, 512], f32)
                    for cc in range(4):
                        chunk = half * 4 + cc
                        nc.tensor.transpose(
                            xt_ps[:, cc * 128:(cc + 1) * 128],
                            x_group[:, j, chunk * 128:(chunk + 1) * 128],
                            ident,
                        )
                    src = xt_ps[:].rearrange("p (c t) -> p c t", c=4)
                    dst = (slice(None), slice(half * 4, (half + 1) * 4),
                           slice(j * 128, (j + 1) * 128))
                    nc.scalar.copy(out=xT[dst[0], dst[1], dst[2]], in_=src)
                    if j == 0 and half == 0:
                        # first unit's square on the idle GPSIMD (runs in
                        # parallel with the remaining ACT copies)
                        nc.gpsimd.tensor_tensor(
                            out=sqT[dst[0], dst[1], dst[2]],
                            in0=xT[dst[0], dst[1], dst[2]],
                            in1=xT[dst[0], dst[1], dst[2]], op=ALU.mult)
                    else:
                        nc.scalar.square(out=sqT[dst[0], dst[1], dst[2]],
                                         in_=xT[dst[0], dst[1], dst[2]])

            # stats: S12[:,0,:] = per-(block,grade) mean, S12[:,1,:] = mean of squares
            S12 = ps_stats.tile([36, 2, GROUP_T], f32)
            for h in range(8):
                nc.tensor.matmul(
                    S12[:, 0, :], gmean_sb[:, h, :], xT[:, h, :],
                    start=(h == 0), stop=(h == 7),
                )
            for h in range(8):
                nc.tensor.matmul(
                    S12[:, 1, :], gmean_sb[:, h, :], sqT[:, h, :],
                    start=(h == 0), stop=(h == 7),
                )

            stats_sb = small_pool.tile([36, 2, GROUP_T], f32)
            nc.scalar.copy(out=stats_sb, in_=S12)
            mean_sb = stats_sb[:, 0, :]
            mean2 = small_pool.tile([36, GROUP_T], f32)
            nc.gpsimd.tensor_tensor(out=mean2, in0=mean_sb, in1=mean_sb,
                                    op=ALU.mult)

            # var = ms - mean^2 (all SBUF, on the idle GPSIMD)
            var_t = small_pool.tile([36, GROUP_T], f32)
            nc.gpsimd.tensor_tensor(out=var_t, in0=stats_sb[:, 1, :],
                                    in1=mean2, op=ALU.subtract)
            # rstd = 1/sqrt(|var + eps|): abs also absorbs tiny negative var
            # from f32r rounding (count-1 grades are masked out anyway)
            rstd_t = small_pool.tile([36, GROUP_T], fmm)
            nc.scalar.activation(rstd_t, var_t, AF.Abs_reciprocal_sqrt,
                                 bias=eps36, scale=1.0)
            c_t = small_pool.tile([36, GROUP_T], f32)
            nc.gpsimd.tensor_tensor(out=c_t, in0=mean_sb, in1=rstd_t,
                                    op=ALU.mult)
            # c2n = b/w - mean*rstd
            c2n_t = small_pool.tile([36, GROUP_T], fmm)
            nc.gpsimd.tensor_scalar(
                out=c2n_t, in0=c_t, scalar1=bw36, scalar2=-1.0,
                op0=ALU.subtract, op1=ALU.mult,
            )

            for j in range(2):
                lhsA = rstd_t[:, j * 128:(j + 1) * 128]
                lhsB = c2n_t[:, j * 128:(j + 1) * 128]
                for half in range(2):
                    sl = slice(half * 512, (half + 1) * 512)
                    b_ps = ps_b.tile([128, 512], f32)
                    a_ps = ps_a.tile([128, 512], f32)
                    nc.tensor.matmul(b_ps, lhsB, ga_sb[:, sl])
                    nc.tensor.matmul(a_ps, lhsA, ga_mask[:, sl])
                    tmp = tmp_pool.tile([128, 512], f32)
                    nc.vector.scalar_tensor_tensor(
                        out=tmp, in0=x_group[:, j, sl], scalar=1.0, in1=a_ps,
                        op0=ALU.mult, op1=ALU.mult)
                    nc.vector.scalar_tensor_tensor(
                        out=x_group[:, j, sl], in0=tmp, scalar=1.0, in1=b_ps,
                        op0=ALU.mult, op1=ALU.add)

            nc.sync.dma_start(
                out=out_d[tok0:tok0 + GROUP_T, :].rearrange("(j p) d -> p j d", p=128),
                in_=x_group,
            )

    if rep_ctx is not None:
        rep_ctx.__exit__(None, None, None)
    nc.finalize()
    return nc


_NC_CACHE = {}


def _get_nc(tok_per_core=TOK_PER_CORE):
    key = (tok_per_core, USE_F32R)
    if key not in _NC_CACHE:
        _NC_CACHE[key] = build_nc(tok_per_core)
    return _NC_CACHE[key]


def kernel(x, weight, bias, _trace=False):
    x = np.ascontiguousarray(np.asarray(x, dtype=np.float32))
    weight = np.ascontiguousarray(np.asarray(weight, dtype=np.float32))
    bias = np.ascontiguousarray(np.asarray(bias, dtype=np.float32))
    orig_shape = x.shape
    xf = x.reshape(TOTAL_TOKENS, D)

    nc = _get_nc()
    from concourse.bass_utils import run_bass_kernel_spmd

    in_maps = [
        {
            "x": np.ascontiguousarray(xf[i * TOK_PER_CORE:(i + 1) * TOK_PER_CORE]),
            "weight": weight,
            "bias": bias,
        }
        for i in range(N_CORES)
    ]
    res = run_bass_kernel_spmd(nc, in_maps, core_ids=list(range(N_CORES)),
                               trace=_trace)
    out = np.concatenate([r["out"] for r in res.results], axis=0)
    if _trace:
        kernel.last_result = res
    return out.reshape(orig_shape)



# revision 4
# speedup vs baseline: 1.4157x; 1.4157x over previous
"""CliffordLayerNorm Trainium2 kernel (v2 — engine-rebalanced).

x: [16, 4096, 1024] fp32. Each row's 1024 features = 4 blocks of 256
multivector components; components are grouped into 9 grades by popcount of
their index within the block.  Per (token, block, grade): mean/var, then
out = (x - mean) * w[g] * rsqrt(var + eps) + b[g].

Data-parallel over tokens across 8 cores (8192 tokens/core), groups of 256
tokens per stats round, software-pipelined two groups deep:

  PE    : 16 transposes -> PSUM (2 rotating 1-bank quarter tiles),
          16 bf16 stats matmuls (grade sums of x and x^2),
          8 f32r scatter matmuls (N=512) expanding per-(block,grade) stats to
          per-element scale A and shift B; the shift matmul carries an
          augmented ones-row so the bias lands in the same matmul.
  ACT   : PSUM->SBUF bf16 copies of x^T, plus the small stats chain
          (S12 copy, mean^2, rsqrt).
  GPSIMD: squares (bf16, SBUF->SBUF) and two small [36,256] stats ops.
  DVE   : only the 2-pass apply (tmp = x*A; out = tmp + B, in place).
  DMA   : 1 MB in + 1 MB out per group (the roofline, ~6 us/group).

PSUM budget is exactly 8 banks: 2 transpose quarters + 2 stats + 2 A + 2 B.
"""

import os
import sys

if "/opt/trn_rl_repo" not in sys.path:
    sys.path.insert(0, "/opt/trn_rl_repo")

import numpy as np

BLOCK_BITS = 8
MV = 256
NG = 9
NB = 4
D = 1024
EPS = 1e-5
N_CORES = 8
TOTAL_TOKENS = 16 * 4096
TOK_PER_CORE = TOTAL_TOKENS // N_CORES  # 8192

GROUP_T = 256          # tokens per stats group
N_CHUNKS = 8           # 128-feature chunks per token row


def _grade(m):
    return bin(m).count("1")


def _build_consts():
    import math
    counts = np.array([math.comb(8, g) for g in range(NG)], dtype=np.float32)

    # gmean[h][p, b*9+g] = 1/count_g for chunk h (features 128h..128h+127)
    gmean = np.zeros((N_CHUNKS, 128, 36), dtype=np.float32)
    for h in range(N_CHUNKS):
        b = h // 2
        for p in range(128):
            m = (h % 2) * 128 + p
            g = _grade(m)
            gmean[h, p, b * 9 + g] = 1.0 / counts[g]

    # g01[b*9+g, c] = 1 if feature c belongs to (block b, grade g)
    g01 = np.zeros((36, D), dtype=np.float32)
    for c in range(D):
        b = c // MV
        g = _grade(c % MV)
        g01[b * 9 + g, c] = 1.0

    # count-1 grades (0 and 8) have centered value exactly 0 in the
    # reference; force their scale A to 0 (mask) and rstd ~ 0 (huge eps)
    # so out = b exactly for those components.
    mask = np.ones((36, 1), dtype=np.float32)
    for b in range(NB):
        mask[b * 9 + 0, 0] = 0.0
        mask[b * 9 + 8, 0] = 0.0
    return gmean, g01, mask


def build_nc(tok_per_core=TOK_PER_CORE):
    import concourse.bass as bass
    import concourse.tile as tile
    from concourse import bacc, mybir

    f32 = mybir.dt.float32
    f32r = mybir.dt.float32r
    bf16 = mybir.dt.bfloat16
    AF = mybir.ActivationFunctionType
    ALU = mybir.AluOpType

    gmean_np, g01_np, mask_np = _build_consts()
    n_groups = tok_per_core // GROUP_T
    assert tok_per_core % GROUP_T == 0

    nc = bacc.Bacc()
    x_d = nc.dram_tensor("x", [tok_per_core, D], f32, kind="ExternalInput")
    w_d = nc.dram_tensor("weight", [NG], f32, kind="ExternalInput")
    b_d = nc.dram_tensor("bias", [NG], f32, kind="ExternalInput")
    out_d = nc.dram_tensor("out", [tok_per_core, D], f32, kind="ExternalOutput")

    gmean_dram = nc.inline_tensor(gmean_np, name="gmean_const")
    g01_dram = nc.inline_tensor(g01_np, name="g01_const")
    ident_dram = nc.inline_tensor(np.eye(128, dtype=np.float32), name="ident_const")
    mask_dram = nc.inline_tensor(mask_np, name="mask_const")
    ones_dram = nc.inline_tensor(np.ones((1, GROUP_T), dtype=np.float32),
                                 name="ones_const")
    diag36_dram = nc.inline_tensor(np.eye(36, dtype=np.float32),
                                   name="diag36_const")

    from contextlib import ExitStack

    with tile.TileContext(nc) as tc, ExitStack() as ctx:
        consts = ctx.enter_context(tc.tile_pool(name="consts", bufs=1))

        # ---- constants into SBUF (single SWDGE queue -> one semaphore) ----
        ident = consts.tile([128, 128], f32)
        nc.gpsimd.dma_start(out=ident, in_=ident_dram[:])

        gmean_bf = consts.tile([128, N_CHUNKS, 36], bf16)
        nc.gpsimd.dma_start(
            out=gmean_bf, in_=gmean_dram[:].rearrange("h p c -> p h c"))

        g01_sb = consts.tile([36, D], f32)
        nc.gpsimd.dma_start(out=g01_sb, in_=g01_dram[:])

        # weight/bias broadcast to 36 partitions: partition b*9+g reads [g]
        w36 = consts.tile([36, 1], f32)
        b36 = consts.tile([36, 1], f32)
        wap = w_d[:]
        bap = b_d[:]
        nc.gpsimd.dma_start(
            out=w36, in_=bass.AP(tensor=wap.tensor, offset=wap.offset,
                                 ap=[[0, NB]] + list(wap.ap)))
        nc.gpsimd.dma_start(
            out=b36, in_=bass.AP(tensor=bap.tensor, offset=bap.offset,
                                 ap=[[0, NB]] + list(bap.ap)))

        mask36 = consts.tile([36, 1], f32)
        nc.gpsimd.dma_start(out=mask36, in_=mask_dram[:])

        # persistent stationary tiles for the shift matmul: rows 0-35 get
        # c = mean*rstd per group, row 36 is the constant 1.0 that pulls the
        # bias row of gB into the same matmul.
        c_t0 = consts.tile([37, GROUP_T], f32r)
        c_t1 = consts.tile([37, GROUP_T], f32r)
        nc.gpsimd.dma_start(out=c_t0[36:37, :], in_=ones_dram[:])
        nc.gpsimd.dma_start(out=c_t1[36:37, :], in_=ones_dram[:])
        c_tiles = [c_t0, c_t1]

        # eps + 1e38*(1-mask): count-1 grades get a huge bias so the fused
        # abs-rsqrt returns ~1e-19 (i.e. rstd ~= 0) for them
        eps36m = consts.tile([36, 1], f32)
        nc.vector.tensor_scalar(
            out=eps36m, in0=mask36, scalar1=-1e38, scalar2=1e38 + EPS,
            op0=ALU.mult, op1=ALU.add)

        # A-matmul moving operand: w[g]*mask*indicator
        w36m = consts.tile([36, 1], f32)
        nc.vector.tensor_scalar_mul(w36m, w36, mask36)
        ga_mask = consts.tile([36, D], f32r)
        nc.vector.tensor_scalar_mul(ga_mask, g01_sb, w36m)

        # B-matmul moving operand: rows 0-35 = -w[g]*indicator, row 36 = b[g(c)].
        # Built whole via PE: stationary [-diag(w) | b] (36x37) against g01,
        # since engine writes must start at a 32-aligned partition.
        diag36_sb = consts.tile([36, 36], f32)
        nc.gpsimd.dma_start(out=diag36_sb, in_=diag36_dram[:])
        gB = consts.tile([37, D], f32r)
        lwb = consts.tile([36, 37], f32)
        nc.vector.tensor_scalar(
            out=lwb[:, 0:36], in0=diag36_sb, scalar1=w36, scalar2=-1.0,
            op0=ALU.mult, op1=ALU.mult)
        nc.vector.tensor_scalar_mul(lwb[:, 36:37], b36, 1.0)

        with tc.tile_pool(name="setup_ps", bufs=1, space="PSUM") as sps:
            bp0 = sps.tile([37, 512], f32)
            bp1 = sps.tile([37, 512], f32)
            nc.tensor.matmul(bp0, lwb, g01_sb[:, 0:512])
            nc.tensor.matmul(bp1, lwb, g01_sb[:, 512:1024])
            nc.scalar.copy(out=gB[:, 0:512], in_=bp0)
            nc.scalar.copy(out=gB[:, 512:1024], in_=bp1)

        # ---- pools ----
        xg_pool = ctx.enter_context(tc.tile_pool(name="xg", bufs=5))
        xts_pool = ctx.enter_context(tc.tile_pool(name="xts", bufs=2))
        sqs_pool = ctx.enter_context(tc.tile_pool(name="sqs", bufs=2))
        tmp_pool = ctx.enter_context(tc.tile_pool(name="tmp", bufs=2))
        small_pool = ctx.enter_context(tc.tile_pool(name="small", bufs=3))
        ps_xt = ctx.enter_context(tc.tile_pool(name="ps_xt", bufs=2, space="PSUM"))
        ps_st = ctx.enter_context(tc.tile_pool(name="ps_st", bufs=2, space="PSUM"))
        ps_a = ctx.enter_context(tc.tile_pool(name="ps_a", bufs=2, space="PSUM"))
        ps_b = ctx.enter_context(tc.tile_pool(name="ps_b", bufs=2, space="PSUM"))

        pending = {}  # gi -> dict(rstd, c_t, x_group, tok0)

        def emit_scatter_tile(st, t):
            """Scatter + apply + (at j boundary) output DMA for tile t=(j,half)
            of a previous group."""
            j, half = t // 2, t % 2
            sl = slice(half * 512, (half + 1) * 512)
            a_ps = ps_a.tile([128, 512], f32)
            b_ps = ps_b.tile([128, 512], f32)
            nc.tensor.matmul(a_ps, st["rstd"][:, j * 128:(j + 1) * 128],
                             ga_mask[:, sl])
            nc.tensor.matmul(b_ps, st["c_t"][:, j * 128:(j + 1) * 128],
                             gB[:, sl])
            tmp = tmp_pool.tile([128, 512], f32)
            xg = st["x_group"]
            nc.vector.tensor_tensor(out=tmp, in0=xg[:, j, sl], in1=a_ps,
                                    op=ALU.mult)
            nc.vector.tensor_tensor(out=xg[:, j, sl], in0=tmp, in1=b_ps,
                                    op=ALU.add)
            if half == 1:
                r0 = st["tok0"] + j * 128
                nc.sync.dma_start(out=out_d[r0:r0 + 128, :], in_=xg[:, j, :])

        def emit_smalls(gi):
            """Small [36, GROUP_T] stats chain for group gi (ACT+GPSIMD)."""
            st = pending[gi]
            S12 = st["S12"]
            stats_sb = small_pool.tile([36, 2, GROUP_T], f32)
            nc.scalar.copy(out=stats_sb, in_=S12)
            m2 = small_pool.tile([36, GROUP_T], f32)
            nc.scalar.square(out=m2, in_=S12[:, 0, :])
            var = small_pool.tile([36, GROUP_T], f32)
            nc.gpsimd.tensor_tensor(out=var, in0=stats_sb[:, 1, :], in1=m2,
                                    op=ALU.subtract)
            rstd = small_pool.tile([36, GROUP_T], f32r)
            nc.scalar.activation(rstd, var, AF.Abs_reciprocal_sqrt,
                                 bias=eps36m, scale=1.0)
            c_t = c_tiles[gi % 2]
            nc.gpsimd.tensor_tensor(out=c_t[0:36, :], in0=stats_sb[:, 0, :],
                                    in1=rstd, op=ALU.mult)
            st["rstd"] = rstd
            st["c_t"] = c_t

        # ---- main loop ----
        for gi in range(n_groups):
            tok0 = gi * GROUP_T
            x_group = xg_pool.tile([128, 2, D], f32)
            nc.sync.dma_start(
                out=x_group,
                in_=x_d[tok0:tok0 + GROUP_T, :].rearrange(
                    "(j p) d -> p j d", p=128),
            )

            xT = xts_pool.tile([128, N_CHUNKS, GROUP_T], bf16)
            sq = sqs_pool.tile([128, N_CHUNKS, GROUP_T], bf16)

            # transposes in 4 one-bank quarters; scatter/apply of group gi-2
            # rides between quarters to plug PE gaps while ACT copies drain
            # the quarter banks.
            for q in range(4):
                xtq = ps_xt.tile([128, 2, GROUP_T], f32)
                for c in range(2):
                    chunk = 2 * q + c
                    for j in range(2):
                        nc.tensor.transpose(
                            xtq[:, c, j * 128:(j + 1) * 128],
                            x_group[:, j, chunk * 128:(chunk + 1) * 128],
                            ident,
                        )
                nc.scalar.copy(out=xT[:, 2 * q:2 * q + 2, :], in_=xtq)
                if q == 1 or q == 3:
                    h = q // 2
                    s = slice(4 * h, 4 * h + 4)
                    nc.gpsimd.tensor_tensor(out=sq[:, s, :], in0=xT[:, s, :],
                                            in1=xT[:, s, :], op=ALU.mult)
                if gi >= 2:
                    emit_scatter_tile(pending[gi - 2], q)

            if gi >= 2:
                del pending[gi - 2]

            # stats: S12[:,0,:] = per-(block,grade) mean, S12[:,1,:] = E[x^2]
            S12 = ps_st.tile([36, 2, GROUP_T], f32)
            for h in range(N_CHUNKS):
                nc.tensor.matmul(S12[:, 0, :], gmean_bf[:, h, :], xT[:, h, :],
                                 start=(h == 0), stop=(h == N_CHUNKS - 1))
            for h in range(N_CHUNKS):
                nc.tensor.matmul(S12[:, 1, :], gmean_bf[:, h, :], sq[:, h, :],
                                 start=(h == 0), stop=(h == N_CHUNKS - 1))

            pending[gi] = {"S12": S12, "x_group": x_group, "tok0": tok0}
            if gi >= 1:
                emit_smalls(gi - 1)

        # drain the two in-flight groups
        for t in range(4):
            emit_scatter_tile(pending[n_groups - 2], t)
        del pending[n_groups - 2]
        emit_smalls(n_groups - 1)
        for t in range(4):
            emit_scatter_tile(pending[n_groups - 1], t)
        del pending[n_groups - 1]

    nc.finalize()
    return nc


_NC_CACHE = {}


def _get_nc(tok_per_core=TOK_PER_CORE):
    key = tok_per_core
    if key not in _NC_CACHE:
        _NC_CACHE[key] = build_nc(tok_per_core)
    return _NC_CACHE[key]


def kernel(x, weight, bias, _trace=False):
    x = np.ascontiguousarray(np.asarray(x, dtype=np.float32))
    weight = np.ascontiguousarray(np.asarray(weight, dtype=np.float32))
    bias = np.ascontiguousarray(np.asarray(bias, dtype=np.float32))
    orig_shape = x.shape
    xf = x.reshape(TOTAL_TOKENS, D)

    nc = _get_nc()
    from concourse.bass_utils import run_bass_kernel_spmd

    in_maps = [
        {
            "x": np.ascontiguousarray(xf[i * TOK_PER_CORE:(i + 1) * TOK_PER_CORE]),
            "weight": weight,
            "bias": bias,
        }
        for i in range(N_CORES)
    ]
    res = run_bass_kernel_spmd(nc, in_maps, core_ids=list(range(N_CORES)),
                               trace=_trace)
    out = np.concatenate([r["out"] for r in res.results], axis=0)
    if _trace:
        kernel.last_result = res
    return out.reshape(orig_shape)
